# revision 25
# baseline (speedup 1.0000x reference)
"""Trainium2 Bass kernel for nn_BiasedMultiHeadAtten (8-core SPMD, tensor
parallel over heads).

The torch module's transpose(0,1)+reshape "scramble" means head n of the
attention only reads rows [64n,64n+64) u [1024+64n,1024+64n+64) of q/k, and
the per-head attention factors into four 1024x1024 score blocks with
contraction 64.  Sharding 2 heads per core therefore also shards the q/k
projections 8-way (256 of 2048 rows each).

Per core c (heads 2c, 2c+1):
  - warmup: 32 dummy matmuls at t=0 flip the PE HAM clock-gate to 8/8
    while the first weight DMAs land; ACT tables (identity/sigmoid/exp)
    preload off the critical path
  - project k then q for its 256 rows (contraction 4096, bf16 PE, fp32
    psum); weight groups stream on two DGE queues (sync/scalar)
  - gated-residual branch for its 256 rows runs on the PE right behind
    the projections (atten_bias arrives in 4 chunks on the vector queue)
  - scrambled attention: S^T = Y^T X per (a0,b0,b1-block), exp on ACT,
    AV via PE with a 64-wide ones block PREPENDED to V^T so the softmax
    denominators come out replicated across psum partitions 0-63 --
    finalize is then just a [64,1024] reciprocal + two muls on DVE
  - normalize + out-proj partial: o_cols @ Wo[:,cols]^T (full 2048 rows)
Host sums the 8 partial outputs with per-core row un-permutation.
"""

import numpy as np
import ml_dtypes

import concourse.bacc as bacc
import concourse.mybir as mybir
import concourse.tile as tile
from concourse import bass_utils

N_CORES = 8
L, H, E, E2, HD = 2048, 1024, 4096, 2048, 64
F32 = mybir.dt.float32
F16 = mybir.dt.float16
BF16 = mybir.dt.bfloat16
I16 = mybir.dt.int16
AF = mybir.ActivationFunctionType
ALU = mybir.AluOpType

# Schraudolph exp -> bf16 bit pattern: i16 = s*0.125*2^7/ln2 + (16256-6)
A_SCH = 0.125 * 128.0 / float(np.log(2.0))
B_SCH = 16250.0

_NC_CACHE = {}


def _perm16(c):
    """Block permutation: device l-tile j holds global l-tile perm[j];
    perm[0] = c and perm[1] = 8 + c so the residual rows sit at tiles 0,1."""
    perm = list(range(16))

    def place(pos, val):
        i = perm.index(val)
        perm[pos], perm[i] = perm[i], perm[pos]

    place(0, c)
    place(1, 8 + c)
    return perm


def _emit(nc, tc, d, out):
    from contextlib import ExitStack

    with ExitStack() as ctx:
        # ---- warmup: PE busy >3.4us to flip HAM warm; ACT table loads ----
        with tc.tile_pool(name="pWm", bufs=1) as pWm, \
             tc.tile_pool(name="psWm", bufs=1, space="PSUM") as psWm:
            wz = pWm.tile([128, 128], BF16, tag="wz", name="wz")
            nc.vector.memset(wz[:], 0)
            ta = pWm.tile([1, 8], F32, tag="ta", name="ta")
            tb = pWm.tile([1, 8], F32, tag="tb", name="tb")
            nc.vector.memset(ta[:], 0)
            nc.scalar.activation(tb[:], ta[:], AF.Identity)
            nc.scalar.activation(tb[:], ta[:], AF.Sigmoid)
            nc.scalar.activation(tb[:], ta[:], AF.Exp)
            wps = psWm.tile([128, 512], F32, tag="wps", name="wps")
            for i in range(32):
                nc.tensor.matmul(wps[:, 128 * (i % 4):128 * (i % 4 + 1)],
                                 wz[:], wz[:], start=True, stop=True)

        pers = ctx.enter_context(tc.tile_pool(name="pers", bufs=1))

        Y = [pers.tile([128, H], BF16, tag=f"Y{b}", name=f"Y{b}")
             for b in range(2)]
        # V^T per b0: 16 j-tiles of [ones64 | V_h0 | ones64 | V_h1] so the
        # AV matmul's psum rows 0-63 accumulate the softmax denominators
        VT = [pers.tile([128, 16, 256], BF16, tag=f"VT{b}", name=f"VT{b}")
              for b in range(2)]
        Ydiag = [[pers.tile([128, 1024], BF16, tag=f"Yd{h}_{b}",
                            name=f"Yd{h}_{b}") for b in range(2)]
                 for h in range(2)]
        Xdup = [[pers.tile([128, 1024], BF16, tag=f"Xd{h}_{a}",
                           name=f"Xd{h}_{a}") for a in range(2)]
                for h in range(2)]

        ident = pers.tile([128, 128], BF16, tag="ident", name="ident")
        nc.gpsimd.dma_start(ident[:], d["ident"][:])
        wo_sb = pers.tile([128, H], BF16, tag="wo", name="wo")
        bias = {}
        rowp = ctx.enter_context(tc.tile_pool(name="rowp", bufs=2))
        for bn in ("bqb", "bkb", "blinb", "bres2b", "bob"):
            row = rowp.tile([1, H], BF16, tag="rowst", name=f"row_{bn}")
            nc.gpsimd.dma_start(row[:], d[bn][:])
            bias[bn] = pers.tile([128, H], BF16, tag=bn, name=bn)
            nc.gpsimd.partition_broadcast(bias[bn][:], row[:])
        # ones blocks of the V^T tiles (constant, written once)
        for b in range(2):
            nc.gpsimd.memset(VT[b][:, :, 0:64], 1.0)
            nc.gpsimd.memset(VT[b][:, :, 128:192], 1.0)

        resg = [pers.tile([128, H], BF16, tag=f"resg{lb}", name=f"resg{lb}")
                for lb in range(2)]
        res1_sb = [pers.tile([128, H], BF16, tag=f"r1s{lb}",
                             name=f"r1s{lb}") for lb in range(2)]
        res1T = [pers.tile([128, 256], BF16, tag=f"r1_{hb}",
                           name=f"r1_{hb}") for hb in range(8)]

        # ================= A: k-pass, q-pass, residual ===================
        with tc.tile_pool(name="phA", bufs=5) as pA, \
             tc.tile_pool(name="phN", bufs=1) as pN, \
             tc.tile_pool(name="phW", bufs=1) as pW, \
             tc.tile_pool(name="psA", bufs=1, space="PSUM") as psA, \
             tc.tile_pool(name="psR", bufs=1, space="PSUM") as psR, \
             tc.tile_pool(name="psT", bufs=2, space="PSUM") as psT:
            nodeT_sb = []
            for g in range(8):
                t = pN.tile([128, 4, 256], BF16, tag=f"node{g}",
                            name=f"node{g}")
                nodeT_sb.append(t)
            nc.sync.dma_start(nodeT_sb[0][:], d["nodeT4"][0])

            ab3 = pW.tile([128, 16, 256], BF16, tag="ab3", name="ab3")

            def node_lhs(e, lb):
                return nodeT_sb[e // 4][:, e % 4, 128 * lb:128 * (lb + 1)]

            kps = [[psA.tile([128, 512], F32, tag=f"pj{lb}{ch}",
                             name=f"k{lb}{ch}") for ch in range(2)]
                   for lb in range(2)]
            for grp in range(16):
                wt = pA.tile([128, 2, H], BF16, tag="wk", name="wk")
                nc.sync.dma_start(wt[:], d["WkT16"][grp])
                if grp < 7:
                    nc.sync.dma_start(nodeT_sb[grp + 1][:],
                                      d["nodeT4"][grp + 1])
                if grp >= 8 and grp % 2 == 0:
                    part = (grp - 8) // 2
                    nc.scalar.dma_start(
                        ab3[:, 4 * part:4 * (part + 1), :],
                        d["abT3"][:, 4 * part:4 * (part + 1), :])
                for e2 in range(2):
                    e = 2 * grp + e2
                    st, sp = (e == 0), (e == 31)
                    for lb in range(2):
                        lhs = node_lhs(e, lb)
                        for ch in range(2):
                            nc.tensor.matmul(
                                kps[lb][ch][:], lhs,
                                wt[:, e2, 512 * ch:512 * (ch + 1)],
                                start=st, stop=sp)
            for lb in range(2):
                for ch in range(2):
                    sl = slice(512 * ch, 512 * (ch + 1))
                    nc.vector.tensor_add(Y[lb][:, sl], kps[lb][ch][:],
                                         bias["bkb"][:, sl])

            # Ydiag: zero-packed score stationaries, built off-PE while the
            # q projection streams (only needs Y; gpsimd queue drains early)
            for h in range(2):
                hp = slice(64 * h, 64 * (h + 1))
                for b0 in range(2):
                    yd = Ydiag[h][b0]
                    eng = nc.vector if b0 == 0 else nc.gpsimd
                    eng.memset(yd[:], 0)
                    src = Y[b0][hp].rearrange("p (j two c) -> p j two c",
                                              two=2, c=64)
                    dst = yd[:].rearrange("p (j two c) -> p j two c",
                                          two=2, c=64)
                    nc.gpsimd.dma_start(dst[0:64, :, 0, :], src[:, :, 0, :])
                    nc.gpsimd.dma_start(dst[64:128, :, 1, :], src[:, :, 1, :])

            # q projection (reuses the kps psum tags); the residual branch
            # operands stream on the vector/gpsimd queues so they arrive
            # progressively before rp1/rp2 need them
            wlin_sb = pW.tile([128, 16, H], BF16, tag="wlin", name="wlin")
            wlin_src = d["WlinT"].rearrange("(q t p) h -> q p t h",
                                            q=4, p=128)
            wlin_dst = wlin_sb[:].rearrange("p (q t) h -> q p t h", q=4)
            wres_sb = [pW.tile([128, 4, H], BF16, tag=f"wres{wg}",
                               name=f"wres{wg}") for wg in range(2)]
            qps = [[psA.tile([128, 512], F32, tag=f"pj{lb}{ch}",
                             name=f"q{lb}{ch}") for ch in range(2)]
                   for lb in range(2)]
            for grp in range(16):
                wt = pA.tile([128, 2, H], BF16, tag="wq", name="wq")
                nc.sync.dma_start(wt[:], d["WqT16"][grp])
                if grp % 4 == 3:
                    nc.sync.dma_start(wlin_dst[grp // 4], wlin_src[grp // 4])
                if grp % 8 == 6:
                    nc.sync.dma_start(wres_sb[grp // 8][:],
                                      d["WresT2"][grp // 8])
                for e2 in range(2):
                    e = 2 * grp + e2
                    st, sp = (e == 0), (e == 31)
                    for lb in range(2):
                        lhs = node_lhs(e, lb)
                        for ch in range(2):
                            nc.tensor.matmul(
                                qps[lb][ch][:], lhs,
                                wt[:, e2, 512 * ch:512 * (ch + 1)],
                                start=st, stop=sp)
            nc.scalar.dma_start(wo_sb[:], d["WoT"][:])

            # Xdup: q rows + bias, duplicated across partition halves
            for a0 in range(2):
                for ch in range(2):
                    sl = slice(512 * ch, 512 * (ch + 1))
                    nc.vector.tensor_add(Xdup[0][a0][0:64, sl],
                                         qps[a0][ch][0:64, :],
                                         bias["bqb"][0:64, sl])
                    nc.vector.tensor_add(Xdup[1][a0][64:128, sl],
                                         qps[a0][ch][64:128, :],
                                         bias["bqb"][64:128, sl])
            for a0 in range(2):
                nc.gpsimd.dma_start(Xdup[0][a0][64:128, :],
                                    Xdup[0][a0][0:64, :])
                nc.gpsimd.dma_start(Xdup[1][a0][0:64, :],
                                    Xdup[1][a0][64:128, :])

            # ---- residual branch: rp1 in two lb passes on 2 psum banks ----
            for lb in range(2):
                p1 = [psR.tile([128, 512], F32, tag=f"r{ch}",
                               name=f"rp1{lb}{ch}") for ch in range(2)]
                for t in range(16):
                    lhs = ab3[:, t, 128 * lb:128 * (lb + 1)]
                    for ch in range(2):
                        nc.tensor.matmul(p1[ch][:], lhs,
                                         wlin_sb[:, t, 512 * ch:512 * (ch + 1)],
                                         start=(t == 0), stop=(t == 15))
                for ch in range(2):
                    sl = slice(512 * ch, 512 * (ch + 1))
                    nc.scalar.activation(res1_sb[lb][:, sl], p1[ch][:],
                                         AF.Identity)

            def emit_vt(b0):
                # V^T tiles for one b0 block: PE transpose + engine copies
                # (ones blocks were memset once at startup)
                for j in range(8):
                    pt = psT.tile([128, 128], BF16, tag="tp", name="tp")
                    nc.tensor.transpose(pt[:], Y[b0][:, 128 * j:128 * (j + 1)],
                                        ident[:])
                    nc.vector.tensor_copy(VT[b0][:, j, 64:128], pt[:, 0:64])
                    nc.vector.tensor_copy(VT[b0][:, j, 192:256], pt[:, 64:128])

            for hb in range(8):
                for lb in range(2):
                    tp = psT.tile([128, 128], BF16, tag="tp", name="tp2")
                    nc.tensor.transpose(tp[:],
                                        res1_sb[lb][:, 128 * hb:128 * (hb + 1)],
                                        ident[:])
                    if hb % 2 == 0:
                        nc.scalar.activation(
                            res1T[hb][:, 128 * lb:128 * (lb + 1)], tp[:],
                            AF.Identity)
                    else:
                        nc.vector.tensor_copy(
                            res1T[hb][:, 128 * lb:128 * (lb + 1)], tp[:])
            emit_vt(0)
            rp2 = [[psA.tile([128, 512], F32, tag=f"pj{lb}{ch}",
                             name=f"rp2{lb}{ch}")
                    for ch in range(2)] for lb in range(2)]
            for wg in range(2):
                for h4 in range(4):
                    hb = 4 * wg + h4
                    for lb in range(2):
                        for ch in range(2):
                            nc.tensor.matmul(
                                rp2[lb][ch][:],
                                res1T[hb][:, 128 * lb:128 * (lb + 1)],
                                wres_sb[wg][:, h4, 512 * ch:512 * (ch + 1)],
                                start=(hb == 0), stop=(hb == 7))
            with tc.tile_pool(name="pG", bufs=1) as pG:
                for lb in range(2):
                    tt = pG.tile([128, H], F32, tag="tt", name=f"tt{lb}")
                    for ch in range(2):
                        sl = slice(512 * ch, 512 * (ch + 1))
                        nc.vector.tensor_add(tt[:, sl], rp2[lb][ch][:],
                                             bias["bres2b"][:, sl])
                    g = pG.tile([128, H], F32, tag="g", name=f"g{lb}")
                    nc.scalar.activation(g[:], tt[:], AF.Sigmoid)
                    nc.gpsimd.tensor_add(resg[lb][:], res1_sb[lb][:],
                                         bias["blinb"][:])
                    nc.gpsimd.tensor_mul(resg[lb][:], resg[lb][:], g[:])
                    nc.gpsimd.tensor_add(resg[lb][:], resg[lb][:],
                                         bias["bob"][:])
            emit_vt(1)

        # ===== C/O shared tiles (allocated after A's pools release) ======
        pCO = ctx.enter_context(tc.tile_pool(name="pCO", bufs=1))
        # parity-blocked, one tile per parity so j<8 out-proj reads never
        # serialize against finalize(1)'s writes (dep tracking is per-tile)
        ocolsT = [pCO.tile([128, 1024], BF16, tag=f"ocolsT{b}",
                           name=f"ocolsT{b}") for b in range(2)]
        rcp_t = [[pCO.tile([64, 1024], F32, tag=f"rcp{a}{h}",
                           name=f"rcp{a}{h}") for h in range(2)]
                 for a in range(2)]

        # ================= C: scrambled attention ========================
        with tc.tile_pool(name="pP", bufs=3) as pP, \
             tc.tile_pool(name="psS", bufs=1, space="PSUM") as psS, \
             tc.tile_pool(name="psO", bufs=1, space="PSUM") as psO:
            O_tiles = [None, None]

            def emit_av(pd):
                pa0, pb0, pj, pbt, pp = pd
                for h in range(2):
                    for ch in range(2):
                        nc.tensor.matmul(
                            O_tiles[pa0][h][:, 512 * ch:512 * (ch + 1)],
                            VT[pb0][:, pj, 128 * h:128 * (h + 1)],
                            pp[h][ch][:],
                            start=(pbt == 0), stop=(pbt == 15))

            def emit_finalize(a0):
                # psum rows 0-63 hold the denominators replicated 64x:
                # reciprocal straight off psum (base partition 0), then
                # normalize psum rows 64-127 (= o) into ocolsT in one mul
                O_ps = O_tiles[a0]
                for h in range(2):
                    nc.vector.reciprocal_approx_fast(rcp_t[a0][h][:],
                                                     O_ps[h][0:64, :])
                for h in range(2):
                    nc.vector.tensor_mul(
                        ocolsT[a0][64 * h:64 * (h + 1), :],
                        O_ps[h][64:128, :], rcp_t[a0][h][:])

            # one flat software pipeline across both parities: AV for
            # iteration i-1 is emitted after the scores of iteration i, so
            # the in-order PE queue never waits on a just-issued exp, and
            # the a0=0 psum drains overlap the a0=1 ramp-up
            pend = None
            for git in range(32):
                a0, bt = git // 16, git % 16
                b0, j = bt // 8, bt % 8
                if bt == 1:
                    # allocate this parity's O psum only after the previous
                    # parity's finalize reads were emitted (correct WAR
                    # tracking), and just before the first AV write lands
                    O_tiles[a0] = [psO.tile([128, 1024], F32, tag=f"O{h}",
                                            name=f"O{h}") for h in range(2)]
                s_ps = [psS.tile([128, 1024], F32, tag=f"s{h}",
                                 name=f"s{h}") for h in range(2)]
                for h in range(2):
                    for ch in range(2):
                        nc.tensor.matmul(
                            s_ps[h][:, 512 * ch:512 * (ch + 1)],
                            Ydiag[h][b0][:, 128 * j:128 * (j + 1)],
                            Xdup[h][a0][:, 512 * ch:512 * (ch + 1)],
                            start=True, stop=True)
                # each head's exp is split 512/512 across ACT and DVE
                # (Schraudolph bitcast) running in parallel, into SEPARATE
                # chunk tiles: dep tracking is whole-tile, so a shared tile
                # would serialize the two writers (WAW) and the s_ps tile
                # would not free until ~1.4us after the scores -- too slow
                # for the next git's score matmuls
                p_sb = [[pP.tile([128, 512], BF16, tag=f"p{h}{ch}",
                                 name=f"p{h}{ch}") for ch in range(2)]
                        for h in range(2)]
                for h in range(2):
                    ca, cd = (0, 1) if h == 0 else (1, 0)
                    nc.scalar.activation(
                        p_sb[h][ca][:],
                        s_ps[h][:, 512 * ca:512 * (ca + 1)],
                        AF.Exp, scale=0.125)
                    nc.vector.tensor_scalar(
                        out=p_sb[h][cd][:].bitcast(I16),
                        in0=s_ps[h][:, 512 * cd:512 * (cd + 1)],
                        scalar1=A_SCH, scalar2=B_SCH,
                        op0=ALU.mult, op1=ALU.add)
                if pend is not None:
                    emit_av(pend)
                    if git == 16:
                        emit_finalize(0)
                pend = (a0, b0, j, bt, p_sb)
            emit_av(pend)
            emit_finalize(1)

        # ================= O: out-projection ============================
        # Parity-blocked: j 0-7 contract ocolsT[:, 0, :] (ready right after
        # finalize(0), mid-C) so their matmuls/converts/DMAs overlap the
        # vector-engine finalize(1); j 8-15 use parity 1.  The residual rows
        # (j = 0 and 8 after the host-side ab row reorder) are folded in via
        # a PE-accumulate with the identity as stationary, so converts never
        # need the vector engine while it drains finalize(1).
        with tc.tile_pool(name="pO", bufs=4) as pO, \
             tc.tile_pool(name="psF", bufs=3, space="PSUM") as psF:
            for j in range(16):
                b, jf = j // 8, j % 8
                op = psF.tile([128, 1024], F32, tag="op", name="op")
                res_j = j % 8 == 0
                for ch in range(2):
                    nc.tensor.matmul(op[:, 512 * ch:512 * (ch + 1)],
                                     ocolsT[b][:, 128 * jf:128 * (jf + 1)],
                                     wo_sb[:, 512 * ch:512 * (ch + 1)],
                                     start=True, stop=not res_j)
                if res_j:
                    for ch in range(2):
                        nc.tensor.matmul(op[:, 512 * ch:512 * (ch + 1)],
                                         ident[:],
                                         resg[b][:, 512 * ch:512 * (ch + 1)],
                                         start=False, stop=True)
                ob = pO.tile([128, H], F16, tag="ob", name="ob")
                # one engine converts the whole tile (a vector+scalar split
                # would serialize: whole-tile WAW tracking), engines
                # alternate per j; j<4 stays on ACT while the vector engine
                # drains finalize(1)
                if j < 4 or j % 2 == 1:
                    nc.scalar.activation(ob[:], op[:], AF.Identity)
                else:
                    nc.vector.tensor_copy(ob[:], op[:])
                eng = nc.sync if j % 2 == 0 else nc.gpsimd
                eng.dma_start(out[128 * j:128 * (j + 1), :], ob[:])


def _build_nc():
    nc = bacc.Bacc("TRN2", target_bir_lowering=False, debug=False,
                   num_devices=N_CORES)
    d = {}

    def din(name, shape, dt=BF16):
        d[name] = nc.dram_tensor(name, shape, dt, kind="ExternalInput").ap()

    din("nodeT4", (8, 128, 4, 256))
    din("WqT16", (16, 128, 2, H))
    din("WkT16", (16, 128, 2, H))
    din("abT3", (128, 16, 256))
    din("WlinT", (E2, H))
    din("WresT2", (2, 128, 4, H))
    din("WoT", (128, H))
    din("ident", (128, 128))
    for bn in ("bqb", "bkb", "blinb", "bres2b", "bob"):
        din(bn, (1, H))
    out = nc.dram_tensor("out", (L, H), F16, kind="ExternalOutput").ap()
    with tile.TileContext(nc) as tc:
        _emit(nc, tc, d, out)
    nc.compile()
    return nc


def get_nc():
    if "nc" not in _NC_CACHE:
        _NC_CACHE["nc"] = _build_nc()
    return _NC_CACHE["nc"]


def build_in_maps(inputs):
    f32 = np.float32
    bf16 = ml_dtypes.bfloat16
    ne = np.asarray(inputs["node_embedding"], f32)
    ab = np.asarray(inputs["atten_bias"], f32)
    Wq = np.asarray(inputs["Wq"], f32)
    Wk = np.asarray(inputs["Wk"], f32)
    Wlin = np.asarray(inputs["Wlin"], f32)
    Wres = np.asarray(inputs["Wres"], f32)
    Wo = np.asarray(inputs["Wo"], f32)
    bq = np.asarray(inputs["bq"], f32)
    bk = np.asarray(inputs["bk"], f32)
    blin = np.asarray(inputs["blin"], f32)
    bres = np.asarray(inputs["bres"], f32)
    bo = np.asarray(inputs["bo"], f32)

    WkT16 = np.ascontiguousarray(
        Wk.T.reshape(16, 2, 128, H).transpose(0, 2, 1, 3)).astype(bf16)
    WlinT = np.ascontiguousarray(Wlin.T).astype(bf16)
    WresT2 = np.ascontiguousarray(
        Wres.T.reshape(2, 4, 128, H).transpose(0, 2, 1, 3)).astype(bf16)
    ident = np.eye(128, dtype=f32).astype(bf16)
    bres2 = (Wres @ blin + bres).astype(f32)

    in_maps = []
    for c in range(N_CORES):
        rows = np.r_[128 * c:128 * (c + 1),
                     1024 + 128 * c:1024 + 128 * (c + 1)]
        # residual rows parity-sorted: resg[0] = even global rows of blocks
        # (c, 8+c), resg[1] = odd -- matching the parity-blocked out tiles
        # j=0 / j=8 row-for-row
        ev = 2 * np.arange(64)
        ab_rows = np.r_[128 * c + ev, 1024 + 128 * c + ev,
                        128 * c + ev + 1, 1024 + 128 * c + ev + 1]
        colperm = np.concatenate([np.arange(64) + 64 * p for p in _perm16(c)])
        in_maps.append({
            "nodeT4": np.ascontiguousarray(
                ne[rows].T.reshape(8, 4, 128, 256).transpose(
                    0, 2, 1, 3)).astype(bf16),
            "WqT16": np.ascontiguousarray(
                Wq.T[:, colperm].reshape(16, 2, 128, H).transpose(
                    0, 2, 1, 3)).astype(bf16),
            "WkT16": WkT16,
            "abT3": np.ascontiguousarray(
                ab[ab_rows].T.reshape(16, 128, 256).transpose(
                    1, 0, 2)).astype(bf16),
            "WlinT": WlinT,
            "WresT2": WresT2,
            "WoT": np.ascontiguousarray(
                Wo[:, 128 * c:128 * (c + 1)].T).astype(bf16),
            "ident": ident,
            "bqb": bq[colperm].reshape(1, H).astype(bf16),
            "bkb": bk.reshape(1, H).astype(bf16),
            "blinb": blin.reshape(1, H).astype(bf16),
            "bres2b": bres2.reshape(1, H).astype(bf16),
            "bob": bo.reshape(1, H).astype(bf16),
        })
    return in_maps


def combine_outputs(results):
    # device out row r = 1024*b + f_dev  (b = qpos parity, f_dev = device
    # feature index); global row l = 2*f_glob + b with f_glob via _perm16
    full = np.zeros((L, H), np.float32)
    fd = np.arange(1024)
    for c in range(N_CORES):
        o = np.asarray(results[c]["out"], np.float32)
        perm = np.asarray(_perm16(c))
        fg = 64 * perm[fd // 64] + fd % 64
        full[2 * fg] += o[0:1024]
        full[2 * fg + 1] += o[1024:2048]
    return full


def kernel(**inputs):
    nc = get_nc()
    in_maps = build_in_maps(inputs)
    res = bass_utils.run_bass_kernel_spmd(nc, in_maps,
                                          core_ids=list(range(N_CORES)))
    return combine_outputs(res.results)


# revision 29
# speedup vs baseline: 1.0727x; 1.0727x over previous
"""Trainium2 Bass kernel for nn_BiasedMultiHeadAtten (8-core SPMD, tensor
parallel over heads).

The torch module's transpose(0,1)+reshape "scramble" means head n of the
attention only reads rows [64n,64n+64) u [1024+64n,1024+64n+64) of q/k, and
the per-head attention factors into four 1024x1024 score blocks with
contraction 64.  Sharding 2 heads per core therefore also shards the q/k
projections 8-way (256 of 2048 rows each).

Per core c (heads 2c, 2c+1):
  - warmup: 32 dummy matmuls at t=0 flip the PE HAM clock-gate to 8/8
    while the first weight DMAs land; ACT tables (identity/sigmoid/exp)
    preload off the critical path
  - project k then q for its 256 rows (contraction 4096, bf16 PE, fp32
    psum); weight groups stream on two DGE queues (sync/scalar)
  - gated-residual branch for its 256 rows runs on the PE right behind
    the projections (atten_bias arrives in 4 chunks on the vector queue)
  - scrambled attention: S^T = Y^T X per (a0,b0,b1-block), exp on ACT,
    AV via PE with a 64-wide ones block PREPENDED to V^T so the softmax
    denominators come out replicated across psum partitions 0-63 --
    finalize is then just a [64,1024] reciprocal + two muls on DVE
  - normalize + out-proj partial: o_cols @ Wo[:,cols]^T (full 2048 rows)
Host sums the 8 partial outputs with per-core row un-permutation.
"""

import numpy as np
import ml_dtypes

import concourse.bacc as bacc
import concourse.mybir as mybir
import concourse.tile as tile
from concourse import bass_utils

N_CORES = 8
L, H, E, E2, HD = 2048, 1024, 4096, 2048, 64
F32 = mybir.dt.float32
F16 = mybir.dt.float16
BF16 = mybir.dt.bfloat16
I16 = mybir.dt.int16
AF = mybir.ActivationFunctionType
ALU = mybir.AluOpType

# Schraudolph exp -> bf16 bit pattern: i16 = s*0.125*2^7/ln2 + (16256-6)
A_SCH = 0.125 * 128.0 / float(np.log(2.0))
B_SCH = 16250.0

_NC_CACHE = {}


def _perm16(c):
    """Block permutation: device l-tile j holds global l-tile perm[j];
    perm[0] = c and perm[1] = 8 + c so the residual rows sit at tiles 0,1."""
    perm = list(range(16))

    def place(pos, val):
        i = perm.index(val)
        perm[pos], perm[i] = perm[i], perm[pos]

    place(0, c)
    place(1, 8 + c)
    return perm


def _emit(nc, tc, d, out):
    from contextlib import ExitStack

    with ExitStack() as ctx:
        # ---- warmup: PE busy >3.4us to flip HAM warm; ACT table loads ----
        with tc.tile_pool(name="pWm", bufs=1) as pWm, \
             tc.tile_pool(name="psWm", bufs=1, space="PSUM") as psWm:
            wz = pWm.tile([128, 128], BF16, tag="wz", name="wz")
            nc.vector.memset(wz[:], 0)
            ta = pWm.tile([1, 8], F32, tag="ta", name="ta")
            tb = pWm.tile([1, 8], F32, tag="tb", name="tb")
            nc.vector.memset(ta[:], 0)
            nc.scalar.activation(tb[:], ta[:], AF.Identity)
            nc.scalar.activation(tb[:], ta[:], AF.Sigmoid)
            nc.scalar.activation(tb[:], ta[:], AF.Exp)
            wps = psWm.tile([128, 512], F32, tag="wps", name="wps")
            for i in range(32):
                nc.tensor.matmul(wps[:, 128 * (i % 4):128 * (i % 4 + 1)],
                                 wz[:], wz[:], start=True, stop=True)

        pers = ctx.enter_context(tc.tile_pool(name="pers", bufs=1))

        Y = [pers.tile([128, H], BF16, tag=f"Y{b}", name=f"Y{b}")
             for b in range(2)]
        # V^T per b0: 16 j-tiles of [ones64 | V_h0 | ones64 | V_h1] so the
        # AV matmul's psum rows 0-63 accumulate the softmax denominators
        VT = [pers.tile([128, 16, 256], BF16, tag=f"VT{b}", name=f"VT{b}")
              for b in range(2)]
        Ydiag = [[pers.tile([128, 1024], BF16, tag=f"Yd{h}_{b}",
                            name=f"Yd{h}_{b}") for b in range(2)]
                 for h in range(2)]
        Xdup = [[pers.tile([128, 1024], BF16, tag=f"Xd{h}_{a}",
                           name=f"Xd{h}_{a}") for a in range(2)]
                for h in range(2)]

        ident = pers.tile([128, 128], BF16, tag="ident", name="ident")
        nc.gpsimd.dma_start(ident[:], d["ident"][:])
        wo_sb = pers.tile([128, H], BF16, tag="wo", name="wo")
        bias = {}
        rowp = ctx.enter_context(tc.tile_pool(name="rowp", bufs=2))
        for bn in ("bqb", "bkb", "blinb", "bres2b", "bob"):
            row = rowp.tile([1, H], BF16, tag="rowst", name=f"row_{bn}")
            nc.gpsimd.dma_start(row[:], d[bn][:])
            bias[bn] = pers.tile([128, H], BF16, tag=bn, name=bn)
            nc.gpsimd.partition_broadcast(bias[bn][:], row[:])
        # ones blocks of the V^T tiles (constant, written once)
        for b in range(2):
            nc.gpsimd.memset(VT[b][:, :, 0:64], 1.0)
            nc.gpsimd.memset(VT[b][:, :, 128:192], 1.0)

        resg = [pers.tile([128, H], BF16, tag=f"resg{lb}", name=f"resg{lb}")
                for lb in range(2)]
        res1_sb = [pers.tile([128, H], BF16, tag=f"r1s{lb}",
                             name=f"r1s{lb}") for lb in range(2)]
        res1T = [pers.tile([128, 256], BF16, tag=f"r1_{hb}",
                           name=f"r1_{hb}") for hb in range(8)]

        # ================= A: k-pass, q-pass, residual ===================
        with tc.tile_pool(name="phA", bufs=5) as pA, \
             tc.tile_pool(name="phN", bufs=1) as pN, \
             tc.tile_pool(name="phW", bufs=1) as pW, \
             tc.tile_pool(name="psA", bufs=1, space="PSUM") as psA, \
             tc.tile_pool(name="psR", bufs=1, space="PSUM") as psR, \
             tc.tile_pool(name="psT", bufs=2, space="PSUM") as psT:
            nodeT_sb = []
            for g in range(8):
                t = pN.tile([128, 4, 256], BF16, tag=f"node{g}",
                            name=f"node{g}")
                nodeT_sb.append(t)
            nc.sync.dma_start(nodeT_sb[0][:], d["nodeT4"][0])

            ab3 = pW.tile([128, 16, 256], BF16, tag="ab3", name="ab3")

            def node_lhs(e, lb):
                return nodeT_sb[e // 4][:, e % 4, 128 * lb:128 * (lb + 1)]

            kps = [[psA.tile([128, 512], F32, tag=f"pj{lb}{ch}",
                             name=f"k{lb}{ch}") for ch in range(2)]
                   for lb in range(2)]
            for grp in range(16):
                wt = pA.tile([128, 2, H], BF16, tag="wk", name="wk")
                nc.sync.dma_start(wt[:], d["WkT16"][grp])
                if grp < 7:
                    nc.sync.dma_start(nodeT_sb[grp + 1][:],
                                      d["nodeT4"][grp + 1])
                if grp >= 8 and grp % 2 == 0:
                    part = (grp - 8) // 2
                    nc.scalar.dma_start(
                        ab3[:, 4 * part:4 * (part + 1), :],
                        d["abT3"][:, 4 * part:4 * (part + 1), :])
                for e2 in range(2):
                    e = 2 * grp + e2
                    st, sp = (e == 0), (e == 31)
                    for lb in range(2):
                        lhs = node_lhs(e, lb)
                        for ch in range(2):
                            nc.tensor.matmul(
                                kps[lb][ch][:], lhs,
                                wt[:, e2, 512 * ch:512 * (ch + 1)],
                                start=st, stop=sp)
            for lb in range(2):
                for ch in range(2):
                    sl = slice(512 * ch, 512 * (ch + 1))
                    nc.vector.tensor_add(Y[lb][:, sl], kps[lb][ch][:],
                                         bias["bkb"][:, sl])

            # Ydiag: zero-packed score stationaries, built off-PE while the
            # q projection streams (only needs Y; gpsimd queue drains early)
            for h in range(2):
                hp = slice(64 * h, 64 * (h + 1))
                for b0 in range(2):
                    yd = Ydiag[h][b0]
                    eng = nc.vector if b0 == 0 else nc.gpsimd
                    eng.memset(yd[:], 0)
                    src = Y[b0][hp].rearrange("p (j two c) -> p j two c",
                                              two=2, c=64)
                    dst = yd[:].rearrange("p (j two c) -> p j two c",
                                          two=2, c=64)
                    nc.gpsimd.dma_start(dst[0:64, :, 0, :], src[:, :, 0, :])
                    nc.gpsimd.dma_start(dst[64:128, :, 1, :], src[:, :, 1, :])

            # q projection (reuses the kps psum tags); the residual branch
            # operands stream on the vector/gpsimd queues so they arrive
            # progressively before rp1/rp2 need them
            wlin_sb = pW.tile([128, 16, H], BF16, tag="wlin", name="wlin")
            wlin_src = d["WlinT"].rearrange("(q t p) h -> q p t h",
                                            q=4, p=128)
            wlin_dst = wlin_sb[:].rearrange("p (q t) h -> q p t h", q=4)
            wres_sb = [pW.tile([128, 4, H], BF16, tag=f"wres{wg}",
                               name=f"wres{wg}") for wg in range(2)]
            qps = [[psA.tile([128, 512], F32, tag=f"pj{lb}{ch}",
                             name=f"q{lb}{ch}") for ch in range(2)]
                   for lb in range(2)]
            for grp in range(16):
                wt = pA.tile([128, 2, H], BF16, tag="wq", name="wq")
                nc.sync.dma_start(wt[:], d["WqT16"][grp])
                if grp % 4 == 3:
                    nc.sync.dma_start(wlin_dst[grp // 4], wlin_src[grp // 4])
                if grp % 8 == 6:
                    nc.sync.dma_start(wres_sb[grp // 8][:],
                                      d["WresT2"][grp // 8])
                for e2 in range(2):
                    e = 2 * grp + e2
                    st, sp = (e == 0), (e == 31)
                    for lb in range(2):
                        lhs = node_lhs(e, lb)
                        for ch in range(2):
                            nc.tensor.matmul(
                                qps[lb][ch][:], lhs,
                                wt[:, e2, 512 * ch:512 * (ch + 1)],
                                start=st, stop=sp)
            nc.scalar.dma_start(wo_sb[:], d["WoT"][:])

            # Xdup: q rows + bias, duplicated across partition halves
            for a0 in range(2):
                for ch in range(2):
                    sl = slice(512 * ch, 512 * (ch + 1))
                    nc.vector.tensor_add(Xdup[0][a0][0:64, sl],
                                         qps[a0][ch][0:64, :],
                                         bias["bqb"][0:64, sl])
                    nc.vector.tensor_add(Xdup[1][a0][64:128, sl],
                                         qps[a0][ch][64:128, :],
                                         bias["bqb"][64:128, sl])
            for a0 in range(2):
                nc.gpsimd.dma_start(Xdup[0][a0][64:128, :],
                                    Xdup[0][a0][0:64, :])
                nc.gpsimd.dma_start(Xdup[1][a0][0:64, :],
                                    Xdup[1][a0][64:128, :])

            # ---- residual branch: rp1 in two lb passes on 2 psum banks ----
            for lb in range(2):
                p1 = [psR.tile([128, 512], F32, tag=f"r{ch}",
                               name=f"rp1{lb}{ch}") for ch in range(2)]
                for t in range(16):
                    lhs = ab3[:, t, 128 * lb:128 * (lb + 1)]
                    for ch in range(2):
                        nc.tensor.matmul(p1[ch][:], lhs,
                                         wlin_sb[:, t, 512 * ch:512 * (ch + 1)],
                                         start=(t == 0), stop=(t == 15))
                for ch in range(2):
                    sl = slice(512 * ch, 512 * (ch + 1))
                    nc.scalar.activation(res1_sb[lb][:, sl], p1[ch][:],
                                         AF.Identity)

            def emit_vt(b0):
                # V^T tiles for one b0 block: PE transpose + engine copies
                # (ones blocks were memset once at startup)
                for j in range(8):
                    pt = psT.tile([128, 128], BF16, tag="tp", name="tp")
                    nc.tensor.transpose(pt[:], Y[b0][:, 128 * j:128 * (j + 1)],
                                        ident[:])
                    nc.vector.tensor_copy(VT[b0][:, j, 64:128], pt[:, 0:64])
                    nc.vector.tensor_copy(VT[b0][:, j, 192:256], pt[:, 64:128])

            for hb in range(8):
                for lb in range(2):
                    tp = psT.tile([128, 128], BF16, tag="tp", name="tp2")
                    nc.tensor.transpose(tp[:],
                                        res1_sb[lb][:, 128 * hb:128 * (hb + 1)],
                                        ident[:])
                    if hb % 2 == 0:
                        nc.scalar.activation(
                            res1T[hb][:, 128 * lb:128 * (lb + 1)], tp[:],
                            AF.Identity)
                    else:
                        nc.vector.tensor_copy(
                            res1T[hb][:, 128 * lb:128 * (lb + 1)], tp[:])
            emit_vt(0)
            rp2 = [[psA.tile([128, 512], F32, tag=f"pj{lb}{ch}",
                             name=f"rp2{lb}{ch}")
                    for ch in range(2)] for lb in range(2)]
            for wg in range(2):
                for h4 in range(4):
                    hb = 4 * wg + h4
                    for lb in range(2):
                        for ch in range(2):
                            nc.tensor.matmul(
                                rp2[lb][ch][:],
                                res1T[hb][:, 128 * lb:128 * (lb + 1)],
                                wres_sb[wg][:, h4, 512 * ch:512 * (ch + 1)],
                                start=(hb == 0), stop=(hb == 7))
            with tc.tile_pool(name="pG", bufs=1) as pG:
                for lb in range(2):
                    tt = pG.tile([128, H], F32, tag="tt", name=f"tt{lb}")
                    for ch in range(2):
                        sl = slice(512 * ch, 512 * (ch + 1))
                        nc.vector.tensor_add(tt[:, sl], rp2[lb][ch][:],
                                             bias["bres2b"][:, sl])
                    g = pG.tile([128, H], F32, tag="g", name=f"g{lb}")
                    nc.scalar.activation(g[:], tt[:], AF.Sigmoid)
                    nc.gpsimd.tensor_add(resg[lb][:], res1_sb[lb][:],
                                         bias["blinb"][:])
                    nc.gpsimd.tensor_mul(resg[lb][:], resg[lb][:], g[:])
                    nc.gpsimd.tensor_add(resg[lb][:], resg[lb][:],
                                         bias["bob"][:])
            emit_vt(1)

        # ===== C/O shared tiles (allocated after A's pools release) ======
        pCO = ctx.enter_context(tc.tile_pool(name="pCO", bufs=1))
        # parity-blocked, one tile per parity so j<8 out-proj reads never
        # serialize against finalize(1)'s writes (dep tracking is per-tile)
        ocolsT = [pCO.tile([128, 1024], BF16, tag=f"ocolsT{b}",
                           name=f"ocolsT{b}") for b in range(2)]
        rcp_t = [[pCO.tile([64, 1024], F32, tag=f"rcp{a}{h}",
                           name=f"rcp{a}{h}") for h in range(2)]
                 for a in range(2)]
        o_dr = [[pCO.tile([64, 1024], F32, tag=f"odr{a}{h}",
                          name=f"odr{a}{h}") for h in range(2)]
                for a in range(2)]

        # ================= C: scrambled attention ========================
        with tc.tile_pool(name="pP", bufs=3) as pP, \
             tc.tile_pool(name="psS", bufs=1, space="PSUM") as psS, \
             tc.tile_pool(name="psO", bufs=1, space="PSUM") as psO:
            O_tiles = [None, None]

            def emit_av(pd):
                pa0, pb0, pj, pbt, pp = pd
                for h in range(2):
                    for ch in range(2):
                        nc.tensor.matmul(
                            O_tiles[pa0][h][:, 512 * ch:512 * (ch + 1)],
                            VT[pb0][:, pj, 128 * h:128 * (h + 1)],
                            pp[h][ch][:],
                            start=(pbt == 0), stop=(pbt == 15))

            def emit_finalize(a0):
                # psum rows 0-63 hold the denominators replicated 64x.
                # Three engines share the work so the psum frees fast (the
                # next parity's AV overwrites these banks two gits later):
                # ACT drains the o rows to sbuf, DVE reciprocals the
                # denominators off psum, GpSimd (no psum access, hence the
                # drain) does the normalize mul into ocolsT.
                O_ps = O_tiles[a0]
                for h in range(2):
                    nc.vector.reciprocal_approx_fast(rcp_t[a0][h][:],
                                                     O_ps[h][0:64, :])
                    nc.scalar.activation(o_dr[a0][h][:], O_ps[h][64:128, :],
                                         AF.Identity)
                for h in range(2):
                    nc.gpsimd.tensor_mul(
                        ocolsT[a0][64 * h:64 * (h + 1), :],
                        o_dr[a0][h][:], rcp_t[a0][h][:])

            # one flat software pipeline across both parities: AV for
            # iteration i-1 is emitted after the scores of iteration i, so
            # the in-order PE queue never waits on a just-issued exp, and
            # the a0=0 psum drains overlap the a0=1 ramp-up
            pend = None
            for git in range(32):
                a0, bt = git // 16, git % 16
                b0, j = bt // 8, bt % 8
                if bt == 1:
                    # allocate this parity's O psum only after the previous
                    # parity's finalize reads were emitted (correct WAR
                    # tracking), and just before the first AV write lands
                    O_tiles[a0] = [psO.tile([128, 1024], F32, tag=f"O{h}",
                                            name=f"O{h}") for h in range(2)]
                # scores and probabilities live in [128,512] CHUNK tiles
                # (4 psum banks, 4 sbuf tiles): dep tracking is whole-tile,
                # so chunking turns one 2-MM+full-exp WAR loop per head
                # (~2.1us, slower than the 1.73us of PE work per git) into
                # four independent 1-MM+half-exp loops that overlap freely
                s_ps = [[psS.tile([128, 512], F32, tag=f"s{h}{ch}",
                                  name=f"s{h}{ch}") for ch in range(2)]
                        for h in range(2)]
                for h in range(2):
                    for ch in range(2):
                        nc.tensor.matmul(
                            s_ps[h][ch][:],
                            Ydiag[h][b0][:, 128 * j:128 * (j + 1)],
                            Xdup[h][a0][:, 512 * ch:512 * (ch + 1)],
                            start=True, stop=True)
                p_sb = [[pP.tile([128, 512], BF16, tag=f"p{h}{ch}",
                                 name=f"p{h}{ch}") for ch in range(2)]
                        for h in range(2)]
                for h in range(2):
                    ca, cd = (0, 1) if h == 0 else (1, 0)
                    nc.scalar.activation(p_sb[h][ca][:], s_ps[h][ca][:],
                                         AF.Exp, scale=0.125)
                    nc.vector.tensor_scalar(
                        out=p_sb[h][cd][:].bitcast(I16),
                        in0=s_ps[h][cd][:],
                        scalar1=A_SCH, scalar2=B_SCH,
                        op0=ALU.mult, op1=ALU.add)
                if pend is not None:
                    emit_av(pend)
                    if git == 16:
                        emit_finalize(0)
                pend = (a0, b0, j, bt, p_sb)
            emit_av(pend)
            emit_finalize(1)

        # ================= O: out-projection ============================
        # Parity-blocked: j 0-7 contract ocolsT[:, 0, :] (ready right after
        # finalize(0), mid-C) so their matmuls/converts/DMAs overlap the
        # vector-engine finalize(1); j 8-15 use parity 1.  The residual rows
        # (j = 0 and 8 after the host-side ab row reorder) are folded in via
        # a PE-accumulate with the identity as stationary, so converts never
        # need the vector engine while it drains finalize(1).
        with tc.tile_pool(name="pO", bufs=4) as pO, \
             tc.tile_pool(name="psF", bufs=3, space="PSUM") as psF:
            for j in range(16):
                b, jf = j // 8, j % 8
                op = psF.tile([128, 1024], F32, tag="op", name="op")
                res_j = j % 8 == 0
                for ch in range(2):
                    nc.tensor.matmul(op[:, 512 * ch:512 * (ch + 1)],
                                     ocolsT[b][:, 128 * jf:128 * (jf + 1)],
                                     wo_sb[:, 512 * ch:512 * (ch + 1)],
                                     start=True, stop=not res_j)
                if res_j:
                    for ch in range(2):
                        nc.tensor.matmul(op[:, 512 * ch:512 * (ch + 1)],
                                         ident[:],
                                         resg[b][:, 512 * ch:512 * (ch + 1)],
                                         start=False, stop=True)
                ob = pO.tile([128, H], F16, tag="ob", name="ob")
                # one engine converts the whole tile (a vector+scalar split
                # would serialize: whole-tile WAW tracking), engines
                # alternate per j; j<2 stays on ACT while the vector engine
                # drains finalize(1)'s reciprocals
                if j < 2 or j % 2 == 1:
                    nc.scalar.activation(ob[:], op[:], AF.Identity)
                else:
                    nc.vector.tensor_copy(ob[:], op[:])
                eng = nc.sync if j % 2 == 0 else nc.gpsimd
                eng.dma_start(out[128 * j:128 * (j + 1), :], ob[:])


def _build_nc():
    nc = bacc.Bacc("TRN2", target_bir_lowering=False, debug=False,
                   num_devices=N_CORES)
    d = {}

    def din(name, shape, dt=BF16):
        d[name] = nc.dram_tensor(name, shape, dt, kind="ExternalInput").ap()

    din("nodeT4", (8, 128, 4, 256))
    din("WqT16", (16, 128, 2, H))
    din("WkT16", (16, 128, 2, H))
    din("abT3", (128, 16, 256))
    din("WlinT", (E2, H))
    din("WresT2", (2, 128, 4, H))
    din("WoT", (128, H))
    din("ident", (128, 128))
    for bn in ("bqb", "bkb", "blinb", "bres2b", "bob"):
        din(bn, (1, H))
    out = nc.dram_tensor("out", (L, H), F16, kind="ExternalOutput").ap()
    with tile.TileContext(nc) as tc:
        _emit(nc, tc, d, out)
    nc.compile()
    return nc


def get_nc():
    if "nc" not in _NC_CACHE:
        _NC_CACHE["nc"] = _build_nc()
    return _NC_CACHE["nc"]


def build_in_maps(inputs):
    f32 = np.float32
    bf16 = ml_dtypes.bfloat16
    ne = np.asarray(inputs["node_embedding"], f32)
    ab = np.asarray(inputs["atten_bias"], f32)
    Wq = np.asarray(inputs["Wq"], f32)
    Wk = np.asarray(inputs["Wk"], f32)
    Wlin = np.asarray(inputs["Wlin"], f32)
    Wres = np.asarray(inputs["Wres"], f32)
    Wo = np.asarray(inputs["Wo"], f32)
    bq = np.asarray(inputs["bq"], f32)
    bk = np.asarray(inputs["bk"], f32)
    blin = np.asarray(inputs["blin"], f32)
    bres = np.asarray(inputs["bres"], f32)
    bo = np.asarray(inputs["bo"], f32)

    WkT16 = np.ascontiguousarray(
        Wk.T.reshape(16, 2, 128, H).transpose(0, 2, 1, 3)).astype(bf16)
    WlinT = np.ascontiguousarray(Wlin.T).astype(bf16)
    WresT2 = np.ascontiguousarray(
        Wres.T.reshape(2, 4, 128, H).transpose(0, 2, 1, 3)).astype(bf16)
    ident = np.eye(128, dtype=f32).astype(bf16)
    bres2 = (Wres @ blin + bres).astype(f32)

    in_maps = []
    for c in range(N_CORES):
        rows = np.r_[128 * c:128 * (c + 1),
                     1024 + 128 * c:1024 + 128 * (c + 1)]
        # residual rows parity-sorted: resg[0] = even global rows of blocks
        # (c, 8+c), resg[1] = odd -- matching the parity-blocked out tiles
        # j=0 / j=8 row-for-row
        ev = 2 * np.arange(64)
        ab_rows = np.r_[128 * c + ev, 1024 + 128 * c + ev,
                        128 * c + ev + 1, 1024 + 128 * c + ev + 1]
        colperm = np.concatenate([np.arange(64) + 64 * p for p in _perm16(c)])
        in_maps.append({
            "nodeT4": np.ascontiguousarray(
                ne[rows].T.reshape(8, 4, 128, 256).transpose(
                    0, 2, 1, 3)).astype(bf16),
            "WqT16": np.ascontiguousarray(
                Wq.T[:, colperm].reshape(16, 2, 128, H).transpose(
                    0, 2, 1, 3)).astype(bf16),
            "WkT16": WkT16,
            "abT3": np.ascontiguousarray(
                ab[ab_rows].T.reshape(16, 128, 256).transpose(
                    1, 0, 2)).astype(bf16),
            "WlinT": WlinT,
            "WresT2": WresT2,
            "WoT": np.ascontiguousarray(
                Wo[:, 128 * c:128 * (c + 1)].T).astype(bf16),
            "ident": ident,
            "bqb": bq[colperm].reshape(1, H).astype(bf16),
            "bkb": bk.reshape(1, H).astype(bf16),
            "blinb": blin.reshape(1, H).astype(bf16),
            "bres2b": bres2.reshape(1, H).astype(bf16),
            "bob": bo.reshape(1, H).astype(bf16),
        })
    return in_maps


def combine_outputs(results):
    # device out row r = 1024*b + f_dev  (b = qpos parity, f_dev = device
    # feature index); global row l = 2*f_glob + b with f_glob via _perm16
    full = np.zeros((L, H), np.float32)
    fd = np.arange(1024)
    for c in range(N_CORES):
        o = np.asarray(results[c]["out"], np.float32)
        perm = np.asarray(_perm16(c))
        fg = 64 * perm[fd // 64] + fd % 64
        full[2 * fg] += o[0:1024]
        full[2 * fg + 1] += o[1024:2048]
    return full


def kernel(**inputs):
    nc = get_nc()
    in_maps = build_in_maps(inputs)
    res = bass_utils.run_bass_kernel_spmd(nc, in_maps,
                                          core_ids=list(range(N_CORES)))
    return combine_outputs(res.results)
